# revision 3
# baseline (speedup 1.0000x reference)
"""MultiHeadGAT layer on 8 Trainium2 NeuronCores (Bass/Tile).

Edge-parallel by target node. Per core:
  phase 1: h = X@W+b and attention scores s_i/s_j for its 1/8 node
           slice -> bf16 table rows [h(128, feat-major) | s_i | s_j],
           AllGather -> full 100k-row table.
  phase 2: per 8192-edge batch: dma_gather of 512B table rows by src;
           per 512-edge chunk (node-aligned, tgt-window < 128):
           per-edge s_i via PE one-hot expansion, w = exp(lrelu(s_i+s_j)),
           one-hot segment-sum matmul into PSUM, flushed as a plain
           contiguous write into a per-pass DRAM accumulator.
  final:   out = (sum_p numer_p) / (sum_p denom_p) + h.

The program is built after the host has examined the edges, so chunk
bases / batch shapes are compile-time constants; the only SWDGE
descriptor-generation load left is the unavoidable per-edge row gather.
"""
import numpy as np
import ml_dtypes

import concourse.bass as bass
import concourse.mybir as mybir
from concourse import library_config
from concourse.library_overlay import lower_extended_insts
from concourse.vector_clock import ScopedClock
import bass_rust

bf16 = ml_dtypes.bfloat16

from concourse.tile import TileContext

_LIMIT = 1


def _split_waits_in_ordered(ordered):
    for bb, insts in ordered.items():
        new = []
        for inst in insts:
            si = inst.sync_info
            if si is not None and len(si.on_wait) > _LIMIT:
                waits = list(si.on_wait)
                si.on_wait = waits[:_LIMIT]
                rest = waits[_LIMIT:]
                for j in range(0, len(rest), _LIMIT):
                    new.append(mybir.InstNoOp(
                        name=f"{inst.name}-wsplit-{j}",
                        sync_info=mybir.SyncInfo(on_wait=rest[j:j + _LIMIT], on_update=[]),
                        bass_nofuse=True,
                        engine=inst.engine,
                    ))
            new.append(inst)
        insts[:] = new


class TC(TileContext):
    """TileContext variant for this walrus build: one sync-wait per
    instruction (excess waits spread over preceding nops)."""

    def _lower_ordered_insts(self, ordered):
        _split_waits_in_ordered(ordered)
        return super()._lower_ordered_insts(ordered)

    def _drain_and_barrier(self, tick_clock, wait_clock):
        drain_inst = self.nc.sync.drain()
        wait_clock.add_sem_waits(drain_inst.ins, ScopedClock({None: tick_clock.global_clock}))
        si = drain_inst.ins.sync_info
        if si is not None and len(si.on_wait) > _LIMIT:
            waits = list(si.on_wait)
            si.on_wait = waits[:_LIMIT]
            for w in waits[_LIMIT:]:
                n = self.nc.sync.nop()
                n.ins.sync_info = bass_rust.SyncInfo(on_wait=[w], on_update=[])
        self.nc.all_engine_barrier()
        assert self.sems is not None
        popped = self.nc._tile_sem_poison_stack.pop()
        assert popped is self._sem_poison
        self.nc.clear_and_free_semaphores(list(self.sems.allocated().values()))
        self.nc.all_engine_barrier()


class Cfg:
    def __init__(self, N=100000, E=1600000, FIN=128, H=4, FH=32, ncores=8,
                 K=64, CH=4, NEG=0.2, SCHUNK=25000):
        self.N, self.E, self.FIN, self.H, self.FH = N, E, FIN, H, FH
        self.HOUT = H * FH
        self.ncores = ncores
        self.NPC = N // ncores
        self.SCHUNK = SCHUNK
        self.NPASS = (N + SCHUNK - 1) // SCHUNK
        self.K = K                  # tiles per batch
        self.EPB = K * 128
        self.CH = CH                # tiles per chunk
        self.CSZ = CH * 128
        self.CPB = K // CH
        self.W = 128
        self.NEG = NEG
        self.ROW = 256              # bf16 cols per table row (512B)
        self.VC = self.HOUT + H     # 132
        self.NT1 = (self.NPC + 127) // 128


def build_nc(cfg, plan):
    """plan: dict with NB, NBP, and per-batch lists:
       nidx[b]       gather count (multiple of 2048, 0 => skip batch)
       chunks[b]     list of (base, write_rows) per real chunk
    """
    c = cfg
    NB = plan["NB"]
    f32, b16, i16 = mybir.dt.float32, mybir.dt.bfloat16, mybir.dt.int16
    nc = bass.Bass(num_devices=c.ncores)

    XTC = c.NT1 * 128
    SC136 = c.VC + 4

    xt = nc.declare_dram_parameter("xt", [c.FIN, XTC], f32, isOutput=False)
    waug = nc.declare_dram_parameter("waug", [c.FIN, SC136], f32, isOutput=False)
    baug = nc.declare_dram_parameter("baug", [1, SC136], f32, isOutput=False)
    g_idx = nc.declare_dram_parameter("g_idx", [NB, 128, c.EPB // 16], i16, isOutput=False)
    tl_in = nc.declare_dram_parameter("tl", [NB, 128, c.K], b16, isOutput=False)
    tlr_in = nc.declare_dram_parameter("tlr", [NB, 1, c.EPB], b16, isOutput=False)
    out_ext = nc.declare_dram_parameter("out", [c.NPC, c.HOUT], f32, isOutput=True)

    h_slice = nc.dram_tensor("h_slice", [c.NPC, c.ROW], b16)
    h_table = nc.dram_tensor("h_table", [c.N, c.ROW], b16, addr_space="Shared")
    accs = [nc.dram_tensor(f"acc{p}", [c.NPC, c.VC], f32) for p in range(c.NPASS)]

    iota1_np = np.tile(np.arange(c.W, dtype=np.float32).astype(bf16)[None, :], (128, 1))
    iota1_d = nc.inline_tensor(iota1_np, name="iota1")
    iotac_np = np.arange(128, dtype=np.float32)[:, None]
    iotac_d = nc.inline_tensor(iotac_np, name="iota_col")
    ones_d = nc.inline_tensor(np.ones((1, 128), np.float32), name="ones_row")
    onesb_d = nc.inline_tensor(np.ones((1, 128), np.float32).astype(bf16), name="ones_bf")

    AOP, AFT = mybir.AluOpType, mybir.ActivationFunctionType

    with TC(nc) as tc:
        nc.gpsimd.load_library(library_config.mlp)

        with tc.tile_pool(name="pc", bufs=1) as pc:
            iot = pc.tile([128, c.W], b16)
            nc.sync.dma_start(out=iot[:], in_=iota1_d[:])
            ioc = pc.tile([128, 1], f32)
            nc.sync.dma_start(out=ioc[:], in_=iotac_d[:])
            onesb = pc.tile([1, 128], b16)
            nc.sync.dma_start(out=onesb[:], in_=onesb_d[:])

            # ---------- phase 1: h / s table slice ----------
            with tc.tile_pool(name="p1", bufs=2) as p1, \
                 tc.tile_pool(name="p1c", bufs=1) as p1c, \
                 tc.tile_pool(name="ps1", bufs=2, space="PSUM") as ps1:
                xts = p1c.tile([c.FIN, XTC], f32)
                nc.sync.dma_start(out=xts[:], in_=xt[:])
                wt = p1c.tile([c.FIN, SC136], f32)
                nc.sync.dma_start(out=wt[:], in_=waug[:])
                bt = p1c.tile([1, SC136], f32)
                nc.sync.dma_start(out=bt[:], in_=baug[:])
                onest = p1c.tile([1, 128], f32)
                nc.sync.dma_start(out=onest[:], in_=ones_d[:])

                for t in range(c.NT1):
                    rows = min(128, c.NPC - t * 128)
                    ps = ps1.tile([128, SC136], f32)
                    nc.tensor.matmul(ps[:], xts[:, t * 128:(t + 1) * 128], wt[:],
                                     start=True, stop=False)
                    nc.tensor.matmul(ps[:], onest[:], bt[:], start=False, stop=True)
                    hb = p1.tile([128, c.ROW], b16, tag="hb")
                    nc.vector.tensor_copy(out=hb[:, 0:SC136], in_=ps[:, 0:SC136])
                    nc.vector.memset(hb[:, SC136:], 0.0)
                    nc.sync.dma_start(out=h_slice[t * 128:t * 128 + rows, :],
                                      in_=hb[:rows, :])

            # ---------- AllGather ----------
            nc.gpsimd.collective_compute(
                "AllGather", AOP.bypass,
                replica_groups=[list(range(c.ncores))],
                ins=[h_slice[:]], outs=[h_table[:]],
            )

            # ---------- zero the accumulators ----------
            with tc.tile_pool(name="pz", bufs=1) as pz:
                ztile = pz.tile([128, 8 * c.VC], f32)
                nc.vector.memset(ztile[:], 0.0)
                z3 = ztile[:].rearrange("p (a v) -> p a v", v=c.VC)
                for acc in accs:
                    full = (c.NPC // 1024) * 1024
                    for r in range(0, full, 1024):
                        o3 = acc[r:r + 1024, :].rearrange("(p a) v -> p a v", a=8)
                        nc.scalar.dma_start(out=o3, in_=z3)
                    rem = c.NPC - full
                    if rem:
                        rows8 = rem // 8
                        o3 = acc[full:full + rows8 * 8, :].rearrange(
                            "(p a) v -> p a v", a=8)
                        nc.scalar.dma_start(out=o3, in_=z3[:rows8])
                        for r in range(full + rows8 * 8, c.NPC, 128):
                            rr = min(128, c.NPC - r)
                            nc.scalar.dma_start(out=acc[r:r + rr, :],
                                                in_=ztile[:rr, 0:c.VC])

            # ---------- phase 2 ----------
            with tc.tile_pool(name="p2", bufs=2) as p2, \
                 tc.tile_pool(name="psv", bufs=2, space="PSUM") as psv, \
                 tc.tile_pool(name="psr", bufs=2, space="PSUM") as psr, \
                 tc.tile_pool(name="pss", bufs=2, space="PSUM") as pss:
                nreg = {}
                for bi in range(NB):
                    nidx = plan["nidx"][bi]
                    if nidx == 0:
                        continue
                    chunks = plan["chunks"][bi]
                    nch = len(chunks)
                    p = plan["pass_of"][bi]
                    pbase = p * c.SCHUNK
                    prows = min(c.SCHUNK, c.N - pbase)
                    acc = accs[p]
                    if nidx not in nreg:
                        nreg[nidx] = nc.gpsimd.to_reg(nidx)

                    git = p2.tile([128, c.EPB // 16], i16, tag="git")
                    nc.sync.dma_start(out=git[:], in_=g_idx[bi])
                    tlt = p2.tile([128, c.K], b16, tag="tlt")
                    nc.sync.dma_start(out=tlt[:], in_=tl_in[bi])
                    tlr = p2.tile([1, c.EPB], b16, tag="tlr")
                    nc.sync.dma_start(out=tlr[:], in_=tlr_in[bi])

                    G = p2.tile([128, c.K * c.ROW], b16, tag="G")
                    G3 = G[:].rearrange("p (k r) -> p k r", r=c.ROW)
                    nc.gpsimd.dma_gather(
                        out_ap=G3[:, 0:nidx // 128, :],
                        in_ap=h_table[pbase:pbase + prows, :],
                        idxs_ap=git[:, 0:nidx // 16],
                        num_idxs=nidx, num_idxs_reg=nreg[nidx],
                        elem_size=c.ROW, single_packet=False)

                    # one-hot for segment sum: O[p, k*W + w] = (tl[p,k] == w)
                    O = p2.tile([128, c.K * c.W], b16, tag="O")
                    nk = sum(ct[2] for ct in chunks)
                    O3 = O[:].rearrange("p (k w) -> p k w", w=c.W)
                    io_b = iot[:].unsqueeze(1).broadcast_to([128, nk, c.W])
                    tl_b = tlt[:, 0:nk].unsqueeze(2).broadcast_to([128, nk, c.W])
                    nc.vector.tensor_tensor(out=O3[:, 0:nk, :], in0=io_b, in1=tl_b,
                                            op=AOP.is_equal)

                    V = p2.tile([128, c.K * c.VC], b16, tag="V")
                    V3 = V[:].rearrange("p (k v) -> p k v", v=c.VC)

                    # per-chunk: s_i expansion and w
                    koff = 0
                    for ch in range(nch):
                        base, wrows, tw = chunks[ch]
                        siw = p2.tile([128, 4], b16, tag="siw")
                        nc.sync.dma_start(
                            out=siw[:wrows, :],
                            in_=h_slice[base:base + wrows, 128:132])
                        for st in range(0, tw, 4):
                            stn = min(4, tw - st)
                            e0 = (koff + st) * 128
                            prep = psr.tile([128, 512], f32)
                            nc.tensor.matmul(prep[:, 0:stn * 128], onesb[:],
                                             tlr[0:1, e0:e0 + stn * 128],
                                             start=True, stop=True)
                            otg = p2.tile([128, 512], b16, tag="otg")
                            nc.vector.tensor_scalar(out=otg[:, 0:stn * 128],
                                                    in0=prep[:, 0:stn * 128],
                                                    scalar1=ioc[:], scalar2=None,
                                                    op0=AOP.is_equal)
                            psi = pss.tile([128, 16], f32)
                            for t in range(stn):
                                nc.tensor.matmul(psi[:, t * 4:(t + 1) * 4],
                                                 otg[:, t * 128:(t + 1) * 128],
                                                 siw[:], start=True, stop=True)
                            k0 = koff + st
                            zt = p2.tile([128, 16], b16, tag="zt")
                            z3 = zt[:].rearrange("p (t h) -> p t h", h=4)
                            nc.vector.tensor_tensor(
                                out=z3[:, 0:stn, :],
                                in0=psi[:, 0:stn * 4].rearrange(
                                    "p (t h) -> p t h", h=4),
                                in1=G3[:, k0:k0 + stn, 132:136], op=AOP.add)
                            zt2 = p2.tile([128, 16], b16, tag="zt2")
                            nc.vector.scalar_tensor_tensor(
                                out=zt2[:, 0:stn * 4], in0=zt[:, 0:stn * 4],
                                scalar=c.NEG, in1=zt[:, 0:stn * 4],
                                op0=AOP.mult, op1=AOP.max)
                            nc.scalar.activation(
                                out=V3[:, k0:k0 + stn, 128:132],
                                in_=zt2[:, 0:stn * 4].rearrange(
                                    "p (t h) -> p t h", h=4),
                                func=AFT.Exp)
                        koff += tw

                    # V_h = h(f,hd) * w(hd)
                    g_h = G3[:, 0:nk, 0:128].rearrange("p k (f h) -> p k f h", h=4)
                    v_h = V3[:, 0:nk, 0:128].rearrange("p k (f h) -> p k f h", h=4)
                    w_b = V3[:, 0:nk, 128:132].unsqueeze(2).broadcast_to(
                        [128, nk, 32, 4])
                    nc.vector.tensor_tensor(out=v_h, in0=g_h, in1=w_b, op=AOP.mult)

                    # segment-sum matmuls + plain accumulator writes
                    koff = 0
                    for ch in range(nch):
                        base, wrows, tw = chunks[ch]
                        pv = psv.tile([128, c.VC], f32)
                        for t in range(tw):
                            k = koff + t
                            nc.tensor.matmul(
                                pv[:], O[:, k * c.W:(k + 1) * c.W],
                                V[:, k * c.VC:(k + 1) * c.VC],
                                start=(t == 0), stop=(t == tw - 1))
                        koff += tw
                        stg = p2.tile([128, c.VC], f32, tag="stg")
                        nc.scalar.copy(out=stg[:], in_=pv[:])
                        nc.scalar.dma_start(out=acc[base:base + wrows, :],
                                            in_=stg[:wrows, :])

            # ---------- final: out = sum(numer)/sum(denom) + h ----------
            with tc.tile_pool(name="p3", bufs=3) as p3:
                for t in range(c.NT1):
                    rows = min(128, c.NPC - t * 128)
                    ats = []
                    for p in range(c.NPASS):
                        at = p3.tile([128, c.VC], f32, tag=f"at{p}")
                        nc.sync.dma_start(out=at[:rows, :],
                                          in_=accs[p][t * 128:t * 128 + rows, :])
                        ats.append(at)
                    s01 = p3.tile([128, c.VC], f32, tag="s01")
                    nc.vector.tensor_tensor(out=s01[:], in0=ats[0][:], in1=ats[1][:],
                                            op=AOP.add)
                    s23 = p3.tile([128, c.VC], f32, tag="s23")
                    nc.vector.tensor_tensor(out=s23[:], in0=ats[2][:], in1=ats[3][:],
                                            op=AOP.add)
                    stot = p3.tile([128, c.VC], f32, tag="stot")
                    nc.vector.tensor_tensor(out=stot[:], in0=s01[:], in1=s23[:],
                                            op=AOP.add)
                    hb2 = p3.tile([128, 128], b16, tag="hb2")
                    nc.sync.dma_start(out=hb2[:rows, :],
                                      in_=h_slice[t * 128:t * 128 + rows, 0:128])
                    dmx = p3.tile([128, 4], f32, tag="dmx")
                    nc.vector.tensor_scalar_max(dmx[:], stot[:, 128:132], 1e-30)
                    rec = p3.tile([128, 4], f32, tag="rec")
                    nc.vector.reciprocal(out=rec[:], in_=dmx[:])
                    tmp = p3.tile([128, 128], f32, tag="tmp")
                    nm = stot[:, 0:128].rearrange("p (f h) -> p f h", h=4)
                    rc = rec[:].unsqueeze(1).broadcast_to([128, 32, 4])
                    tw = tmp[:].rearrange("p (h f) -> p f h", h=4)
                    nc.vector.tensor_tensor(out=tw, in0=nm, in1=rc, op=AOP.mult)
                    ot = p3.tile([128, 128], f32, tag="ot")
                    hperm = hb2[:].rearrange("p (f h) -> p h f", h=4)
                    o2 = ot[:].rearrange("p (h f) -> p h f", h=4)
                    t2 = tmp[:].rearrange("p (h f) -> p h f", h=4)
                    nc.vector.tensor_tensor(out=o2, in0=t2, in1=hperm, op=AOP.add)
                    nc.sync.dma_start(out=out_ext[t * 128:t * 128 + rows, :],
                                      in_=ot[:rows, :])

    lower_extended_insts(nc)
    return nc


def wrap16(arr, n):
    w = np.zeros((128, n // 16), arr.dtype)
    w[:16, :] = arr.reshape(n // 16, 16).T
    w[16:, :] = np.tile(w[:16, :], (7, 1))
    return w


def host_prep(cfg, node_features, edge_index, W, b, attn):
    c = cfg
    X = np.asarray(node_features, np.float32)
    W = np.asarray(W, np.float32)
    bb = np.asarray(b, np.float32)
    attn = np.asarray(attn, np.float32)
    tgt = np.asarray(edge_index[0], np.int64)
    src = np.asarray(edge_index[1], np.int64)

    H, FH = c.H, c.FH
    a_i, a_j = attn[0, :, :FH], attn[0, :, FH:]
    Wp = W.reshape(c.FIN, H, FH).transpose(0, 2, 1).reshape(c.FIN, H * FH)
    bp = bb.reshape(H, FH).T.reshape(-1)
    WA_i = np.einsum('ihf,hf->ih', W.reshape(c.FIN, H, FH), a_i)
    WA_j = np.einsum('ihf,hf->ih', W.reshape(c.FIN, H, FH), a_j)
    bA_i = np.einsum('hf,hf->h', bb.reshape(H, FH), a_i)
    bA_j = np.einsum('hf,hf->h', bb.reshape(H, FH), a_j)
    waug = np.concatenate([Wp, WA_i, WA_j], axis=1).astype(np.float32)
    baug = np.concatenate([bp, bA_i, bA_j])[None, :].astype(np.float32)

    core_of = tgt // c.NPC
    NW = (c.NPC + 127) // 128  # fixed 128-node windows
    ecount = np.zeros((c.ncores, c.NPASS, NW), np.int64)
    edges = {}
    for cc in range(c.ncores):
        m = core_of == cc
        tl_g = (tgt[m] - cc * c.NPC).astype(np.int64)
        sr = src[m]
        o = np.argsort(tl_g, kind='stable')
        tl_g, sr = tl_g[o], sr[o]
        for p in range(c.NPASS):
            pm = (sr // c.SCHUNK) == p
            tlp, srp = tl_g[pm], sr[pm] - p * c.SCHUNK
            edges[(cc, p)] = (tlp, srp)
            ecount[cc, p] = np.bincount(tlp // 128, minlength=NW)
    # per-window tile budget shared across cores (max), then pack into batches
    T = np.maximum(ecount, 0).max(axis=0)          # [NPASS, NW] edges max
    T = (T + 127) // 128                            # tiles per window
    batches = []   # list of (pass, [(w, ntiles), ...])
    for p in range(c.NPASS):
        cur, cnt = [], 0
        for w in range(NW):
            tw = int(T[p, w])
            if tw == 0:
                continue
            if cnt + tw > c.K:
                batches.append((p, cur)); cur, cnt = [], 0
            cur.append((w, tw)); cnt += tw
        if cur:
            batches.append((p, cur))
    NB = len(batches)

    plan = {"NB": NB, "pass_of": [p for p, _ in batches], "nidx": [], "chunks": []}
    for p, wins in batches:
        ntile = sum(tw for _, tw in wins)
        plan["nidx"].append(min((ntile * 128 + 2047) // 2048 * 2048, c.EPB))
        chl = []
        for w, tw in wins:
            base = w * 128
            wrows = min(128, c.NPC - base)
            chl.append((base, wrows, tw))
        plan["chunks"].append(chl)

    in_maps = []
    for cc in range(c.ncores):
        g_i = np.zeros((NB, c.EPB), np.int16)
        t_l = np.full((NB, c.EPB), 999.0, np.float32)
        for bi, (p, wins) in enumerate(batches):
            tlp, srp = edges[(cc, p)]
            cur = 0
            for w, tw in wins:
                i0 = int(np.searchsorted(tlp, w * 128, 'left'))
                i1 = int(np.searchsorted(tlp, w * 128 + 128, 'left'))
                ln = i1 - i0
                g_i[bi, cur:cur + ln] = srp[i0:i1].astype(np.int16)
                t_l[bi, cur:cur + ln] = (tlp[i0:i1] - w * 128).astype(np.float32)
                cur += tw * 128
        g_w = np.stack([wrap16(g_i[bn], c.EPB) for bn in range(NB)])
        tl_w = t_l.reshape(NB, c.K, 128).transpose(0, 2, 1).astype(bf16)
        tlr_w = t_l.reshape(NB, 1, c.EPB).astype(bf16)
        xtv = np.zeros((c.FIN, c.NT1 * 128), np.float32)
        xtv[:, :c.NPC] = X[cc * c.NPC:(cc + 1) * c.NPC].T
        in_maps.append({
            "xt": xtv, "waug": waug, "baug": baug,
            "g_idx": g_w, "tl": tl_w, "tlr": tlr_w,
        })
    return in_maps, plan, edges


def reference_np(cfg, node_features, edge_index, W, b, attn):
    c = cfg
    X = np.asarray(node_features, np.float64)
    W_ = np.asarray(W, np.float64)
    b_ = np.asarray(b, np.float64)
    a = np.asarray(attn, np.float64)
    h = (X @ W_ + b_).reshape(c.N, c.H, c.FH)
    tgt, src = np.asarray(edge_index[0]), np.asarray(edge_index[1])
    a_i, a_j = a[0, :, :c.FH], a[0, :, c.FH:]
    s_i = np.einsum('nhf,hf->nh', h, a_i)
    s_j = np.einsum('nhf,hf->nh', h, a_j)
    e = s_i[tgt] + s_j[src]
    e = np.where(e > 0, e, c.NEG * e)
    m = np.full((c.N, c.H), -np.inf)
    np.maximum.at(m, tgt, e)
    ex = np.exp(e - m[tgt])
    den = np.zeros((c.N, c.H))
    np.add.at(den, tgt, ex)
    alpha = ex / den[tgt]
    msg = alpha[:, :, None] * h[src]
    out = np.zeros((c.N, c.H, c.FH))
    np.add.at(out, tgt, msg)
    out = out + h
    return out.reshape(c.N, c.H * c.FH).astype(np.float32)


# ---------------- public entry point ----------------
import os as _os

LAST_EXEC_NS = None


def kernel(**inputs):
    global LAST_EXEC_NS
    from concourse.bass_utils import run_bass_kernel_spmd

    cfg = Cfg()
    node_features = inputs["node_features"]
    edge_index = inputs["edge_index"]
    W, b, attn = inputs["W"], inputs["b"], inputs["attn"]

    in_maps, plan, _ = host_prep(cfg, node_features, edge_index, W, b, attn)
    nc = build_nc(cfg, plan)
    res = run_bass_kernel_spmd(
        nc, in_maps, list(range(cfg.ncores)),
        trace=bool(_os.environ.get("GAT_TRACE")),
    )
    LAST_EXEC_NS = res.exec_time_ns
    out = np.concatenate(
        [np.asarray(res.results[i]["out"]) for i in range(cfg.ncores)], axis=0)
    return np.ascontiguousarray(out, dtype=np.float32)


# revision 4
# speedup vs baseline: 1.0899x; 1.0899x over previous
"""MultiHeadGAT layer on 8 Trainium2 NeuronCores (Bass/Tile).

Edge-parallel by target node. Per core:
  phase 1: h = X@W+b and attention scores s_i/s_j for its 1/8 node
           slice -> bf16 table rows [h(128, feat-major) | s_i | s_j],
           AllGather -> full 100k-row table.
  phase 2: per 8192-edge batch: dma_gather of 512B table rows by src;
           per 512-edge chunk (node-aligned, tgt-window < 128):
           per-edge s_i via PE one-hot expansion, w = exp(lrelu(s_i+s_j)),
           one-hot segment-sum matmul into PSUM, flushed as a plain
           contiguous write into a per-pass DRAM accumulator.
  final:   out = (sum_p numer_p) / (sum_p denom_p) + h.

The program is built after the host has examined the edges, so chunk
bases / batch shapes are compile-time constants; the only SWDGE
descriptor-generation load left is the unavoidable per-edge row gather.
"""
import numpy as np
import ml_dtypes

import concourse.bass as bass
import concourse.mybir as mybir
from concourse import library_config
from concourse.library_overlay import lower_extended_insts
from concourse.vector_clock import ScopedClock
import bass_rust

bf16 = ml_dtypes.bfloat16

from concourse.tile import TileContext

_LIMIT = 1


def _split_waits_in_ordered(ordered):
    for bb, insts in ordered.items():
        new = []
        for inst in insts:
            si = inst.sync_info
            if si is not None and len(si.on_wait) > _LIMIT:
                waits = list(si.on_wait)
                si.on_wait = waits[:_LIMIT]
                rest = waits[_LIMIT:]
                for j in range(0, len(rest), _LIMIT):
                    new.append(mybir.InstNoOp(
                        name=f"{inst.name}-wsplit-{j}",
                        sync_info=mybir.SyncInfo(on_wait=rest[j:j + _LIMIT], on_update=[]),
                        bass_nofuse=True,
                        engine=inst.engine,
                    ))
            new.append(inst)
        insts[:] = new


class TC(TileContext):
    """TileContext variant for this walrus build: one sync-wait per
    instruction (excess waits spread over preceding nops)."""

    def _lower_ordered_insts(self, ordered):
        _split_waits_in_ordered(ordered)
        return super()._lower_ordered_insts(ordered)

    def _drain_and_barrier(self, tick_clock, wait_clock):
        drain_inst = self.nc.sync.drain()
        wait_clock.add_sem_waits(drain_inst.ins, ScopedClock({None: tick_clock.global_clock}))
        si = drain_inst.ins.sync_info
        if si is not None and len(si.on_wait) > _LIMIT:
            waits = list(si.on_wait)
            si.on_wait = waits[:_LIMIT]
            for w in waits[_LIMIT:]:
                n = self.nc.sync.nop()
                n.ins.sync_info = bass_rust.SyncInfo(on_wait=[w], on_update=[])
        self.nc.all_engine_barrier()
        assert self.sems is not None
        popped = self.nc._tile_sem_poison_stack.pop()
        assert popped is self._sem_poison
        self.nc.clear_and_free_semaphores(list(self.sems.allocated().values()))
        self.nc.all_engine_barrier()


class Cfg:
    def __init__(self, N=100000, E=1600000, FIN=128, H=4, FH=32, ncores=8,
                 K=64, CH=4, NEG=0.2, SCHUNK=25000):
        self.N, self.E, self.FIN, self.H, self.FH = N, E, FIN, H, FH
        self.HOUT = H * FH
        self.ncores = ncores
        self.NPC = N // ncores
        self.SCHUNK = SCHUNK
        self.NPASS = (N + SCHUNK - 1) // SCHUNK
        self.K = K                  # tiles per batch
        self.EPB = K * 128
        self.CH = CH                # tiles per chunk
        self.CSZ = CH * 128
        self.CPB = K // CH
        self.W = 128
        self.NEG = NEG
        self.ROW = 256              # bf16 cols per table row (512B)
        self.VC = self.HOUT + H     # 132
        self.NT1 = (self.NPC + 127) // 128


def build_nc(cfg, plan):
    """plan: dict with NB, NBP, and per-batch lists:
       nidx[b]       gather count (multiple of 2048, 0 => skip batch)
       chunks[b]     list of (base, write_rows) per real chunk
    """
    c = cfg
    NB = plan["NB"]
    f32, b16, i16 = mybir.dt.float32, mybir.dt.bfloat16, mybir.dt.int16
    nc = bass.Bass(num_devices=c.ncores)

    XTC = c.NT1 * 128
    SC136 = c.VC + 4

    xt = nc.declare_dram_parameter("xt", [c.FIN, XTC], f32, isOutput=False)
    waug = nc.declare_dram_parameter("waug", [c.FIN, SC136], f32, isOutput=False)
    baug = nc.declare_dram_parameter("baug", [1, SC136], f32, isOutput=False)
    g_idx = nc.declare_dram_parameter("g_idx", [NB, 128, c.EPB // 16], i16, isOutput=False)
    tl_in = nc.declare_dram_parameter("tl", [NB, 128, c.K], b16, isOutput=False)
    tlr_in = nc.declare_dram_parameter("tlr", [NB, 1, c.EPB], b16, isOutput=False)
    out_ext = nc.declare_dram_parameter("out", [c.NPC, c.HOUT], f32, isOutput=True)

    h_slice = nc.dram_tensor("h_slice", [c.NPC, c.ROW], b16)
    h_table = nc.dram_tensor("h_table", [c.N, c.ROW], b16, addr_space="Shared")
    accs = [nc.dram_tensor(f"acc{p}", [c.NPC, c.VC], f32) for p in range(c.NPASS)]

    iota1_np = np.tile(np.arange(c.W, dtype=np.float32).astype(bf16)[None, :], (128, 1))
    iota1_d = nc.inline_tensor(iota1_np, name="iota1")
    iotac_np = np.arange(128, dtype=np.float32)[:, None]
    iotac_d = nc.inline_tensor(iotac_np, name="iota_col")
    ones_d = nc.inline_tensor(np.ones((1, 128), np.float32), name="ones_row")
    onesb_d = nc.inline_tensor(np.ones((1, 128), np.float32).astype(bf16), name="ones_bf")

    AOP, AFT = mybir.AluOpType, mybir.ActivationFunctionType

    with TC(nc) as tc:
        nc.gpsimd.load_library(library_config.mlp)

        with tc.tile_pool(name="pc", bufs=1) as pc:
            iot = pc.tile([128, c.W], b16)
            nc.sync.dma_start(out=iot[:], in_=iota1_d[:])
            ioc = pc.tile([128, 1], f32)
            nc.sync.dma_start(out=ioc[:], in_=iotac_d[:])
            onesb = pc.tile([1, 128], b16)
            nc.sync.dma_start(out=onesb[:], in_=onesb_d[:])

            # ---------- phase 1: h / s table slice ----------
            with tc.tile_pool(name="p1", bufs=2) as p1, \
                 tc.tile_pool(name="p1c", bufs=1) as p1c, \
                 tc.tile_pool(name="ps1", bufs=2, space="PSUM") as ps1:
                xts = p1c.tile([c.FIN, XTC], f32)
                nc.sync.dma_start(out=xts[:], in_=xt[:])
                wt = p1c.tile([c.FIN, SC136], f32)
                nc.sync.dma_start(out=wt[:], in_=waug[:])
                bt = p1c.tile([1, SC136], f32)
                nc.sync.dma_start(out=bt[:], in_=baug[:])
                onest = p1c.tile([1, 128], f32)
                nc.sync.dma_start(out=onest[:], in_=ones_d[:])

                for t in range(c.NT1):
                    rows = min(128, c.NPC - t * 128)
                    ps = ps1.tile([128, SC136], f32)
                    nc.tensor.matmul(ps[:], xts[:, t * 128:(t + 1) * 128], wt[:],
                                     start=True, stop=False)
                    nc.tensor.matmul(ps[:], onest[:], bt[:], start=False, stop=True)
                    hb = p1.tile([128, c.ROW], b16, tag="hb")
                    nc.vector.tensor_copy(out=hb[:, 0:SC136], in_=ps[:, 0:SC136])
                    nc.vector.memset(hb[:, SC136:], 0.0)
                    nc.sync.dma_start(out=h_slice[t * 128:t * 128 + rows, :],
                                      in_=hb[:rows, :])

            # ---------- AllGather ----------
            nc.gpsimd.collective_compute(
                "AllGather", AOP.bypass,
                replica_groups=[list(range(c.ncores))],
                ins=[h_slice[:]], outs=[h_table[:]],
            )

            # ---------- zero the accumulators ----------
            with tc.tile_pool(name="pz", bufs=1) as pz:
                ztile = pz.tile([128, 8 * c.VC], f32)
                nc.vector.memset(ztile[:], 0.0)
                z3 = ztile[:].rearrange("p (a v) -> p a v", v=c.VC)
                for acc in accs:
                    full = (c.NPC // 1024) * 1024
                    for r in range(0, full, 1024):
                        o3 = acc[r:r + 1024, :].rearrange("(p a) v -> p a v", a=8)
                        nc.scalar.dma_start(out=o3, in_=z3)
                    rem = c.NPC - full
                    if rem:
                        rows8 = rem // 8
                        o3 = acc[full:full + rows8 * 8, :].rearrange(
                            "(p a) v -> p a v", a=8)
                        nc.scalar.dma_start(out=o3, in_=z3[:rows8])
                        for r in range(full + rows8 * 8, c.NPC, 128):
                            rr = min(128, c.NPC - r)
                            nc.scalar.dma_start(out=acc[r:r + rr, :],
                                                in_=ztile[:rr, 0:c.VC])

            # ---------- phase 2 ----------
            with tc.tile_pool(name="p2", bufs=2) as p2, \
                 tc.tile_pool(name="psv", bufs=3, space="PSUM") as psv, \
                 tc.tile_pool(name="psr", bufs=2, space="PSUM") as psr, \
                 tc.tile_pool(name="pss", bufs=2, space="PSUM") as pss, \
                 tc.tile_pool(name="p2s", bufs=6) as p2s:
                nreg = {}
                for bi in range(NB):
                    nidx = plan["nidx"][bi]
                    if nidx == 0:
                        continue
                    chunks = plan["chunks"][bi]
                    nch = len(chunks)
                    p = plan["pass_of"][bi]
                    pbase = p * c.SCHUNK
                    prows = min(c.SCHUNK, c.N - pbase)
                    acc = accs[p]
                    if nidx not in nreg:
                        nreg[nidx] = nc.gpsimd.to_reg(nidx)

                    git = p2.tile([128, c.EPB // 16], i16, tag="git")
                    nc.sync.dma_start(out=git[:], in_=g_idx[bi])
                    tlt = p2.tile([128, c.K], b16, tag="tlt")
                    nc.sync.dma_start(out=tlt[:], in_=tl_in[bi])
                    tlr = p2.tile([1, c.EPB], b16, tag="tlr")
                    nc.sync.dma_start(out=tlr[:], in_=tlr_in[bi])

                    G = p2.tile([128, c.K * c.ROW], b16, tag="G")
                    G3 = G[:].rearrange("p (k r) -> p k r", r=c.ROW)
                    nc.gpsimd.dma_gather(
                        out_ap=G3[:, 0:nidx // 128, :],
                        in_ap=h_table[pbase:pbase + prows, :],
                        idxs_ap=git[:, 0:nidx // 16],
                        num_idxs=nidx, num_idxs_reg=nreg[nidx],
                        elem_size=c.ROW, single_packet=False)

                    # one-hot for segment sum: O[p, k*W + w] = (tl[p,k] == w)
                    O = p2.tile([128, c.K * c.W], b16, tag="O")
                    nk = sum(ct[2] for ct in chunks)
                    O3 = O[:].rearrange("p (k w) -> p k w", w=c.W)
                    io_b = iot[:].unsqueeze(1).broadcast_to([128, nk, c.W])
                    tl_b = tlt[:, 0:nk].unsqueeze(2).broadcast_to([128, nk, c.W])
                    nc.vector.tensor_tensor(out=O3[:, 0:nk, :], in0=io_b, in1=tl_b,
                                            op=AOP.is_equal)

                    V = p2.tile([128, c.K * c.VC], b16, tag="V")
                    V3 = V[:].rearrange("p (k v) -> p k v", v=c.VC)

                    # per-chunk: s_i expansion and w
                    koff = 0
                    for ch in range(nch):
                        base, wrows, tw = chunks[ch]
                        siw = p2s.tile([128, 4], b16, tag="siw")
                        nc.sync.dma_start(
                            out=siw[:wrows, :],
                            in_=h_slice[base:base + wrows, 128:132])
                        for st in range(0, tw, 4):
                            stn = min(4, tw - st)
                            e0 = (koff + st) * 128
                            prep = psr.tile([128, 512], f32)
                            nc.tensor.matmul(prep[:, 0:stn * 128], onesb[:],
                                             tlr[0:1, e0:e0 + stn * 128],
                                             start=True, stop=True)
                            otg = p2s.tile([128, 512], b16, tag="otg")
                            nc.vector.tensor_scalar(out=otg[:, 0:stn * 128],
                                                    in0=prep[:, 0:stn * 128],
                                                    scalar1=ioc[:], scalar2=None,
                                                    op0=AOP.is_equal)
                            psi = pss.tile([128, 16], f32)
                            for t in range(stn):
                                nc.tensor.matmul(psi[:, t * 4:(t + 1) * 4],
                                                 otg[:, t * 128:(t + 1) * 128],
                                                 siw[:], start=True, stop=True)
                            k0 = koff + st
                            zt = p2s.tile([128, 16], b16, tag="zt")
                            z3 = zt[:].rearrange("p (t h) -> p t h", h=4)
                            nc.vector.tensor_tensor(
                                out=z3[:, 0:stn, :],
                                in0=psi[:, 0:stn * 4].rearrange(
                                    "p (t h) -> p t h", h=4),
                                in1=G3[:, k0:k0 + stn, 132:136], op=AOP.add)
                            zt2 = p2s.tile([128, 16], b16, tag="zt2")
                            nc.vector.scalar_tensor_tensor(
                                out=zt2[:, 0:stn * 4], in0=zt[:, 0:stn * 4],
                                scalar=c.NEG, in1=zt[:, 0:stn * 4],
                                op0=AOP.mult, op1=AOP.max)
                            nc.scalar.activation(
                                out=V3[:, k0:k0 + stn, 128:132],
                                in_=zt2[:, 0:stn * 4].rearrange(
                                    "p (t h) -> p t h", h=4),
                                func=AFT.Exp)
                        koff += tw

                    # V_h = h(f,hd) * w(hd)
                    g_h = G3[:, 0:nk, 0:128].rearrange("p k (f h) -> p k f h", h=4)
                    v_h = V3[:, 0:nk, 0:128].rearrange("p k (f h) -> p k f h", h=4)
                    w_b = V3[:, 0:nk, 128:132].unsqueeze(2).broadcast_to(
                        [128, nk, 32, 4])
                    nc.vector.tensor_tensor(out=v_h, in0=g_h, in1=w_b, op=AOP.mult)

                    # segment-sum matmuls + plain accumulator writes
                    koff = 0
                    for ch in range(nch):
                        base, wrows, tw = chunks[ch]
                        pv = psv.tile([128, c.VC], f32)
                        for t in range(tw):
                            k = koff + t
                            nc.tensor.matmul(
                                pv[:], O[:, k * c.W:(k + 1) * c.W],
                                V[:, k * c.VC:(k + 1) * c.VC],
                                start=(t == 0), stop=(t == tw - 1))
                        koff += tw
                        stg = p2s.tile([128, c.VC], f32, tag="stg")
                        nc.scalar.copy(out=stg[:], in_=pv[:])
                        nc.scalar.dma_start(out=acc[base:base + wrows, :],
                                            in_=stg[:wrows, :])

            # ---------- final: out = sum(numer)/sum(denom) + h ----------
            with tc.tile_pool(name="p3", bufs=3) as p3:
                for t in range(c.NT1):
                    rows = min(128, c.NPC - t * 128)
                    ats = []
                    for p in range(c.NPASS):
                        at = p3.tile([128, c.VC], f32, tag=f"at{p}")
                        nc.sync.dma_start(out=at[:rows, :],
                                          in_=accs[p][t * 128:t * 128 + rows, :])
                        ats.append(at)
                    s01 = p3.tile([128, c.VC], f32, tag="s01")
                    nc.vector.tensor_tensor(out=s01[:], in0=ats[0][:], in1=ats[1][:],
                                            op=AOP.add)
                    s23 = p3.tile([128, c.VC], f32, tag="s23")
                    nc.vector.tensor_tensor(out=s23[:], in0=ats[2][:], in1=ats[3][:],
                                            op=AOP.add)
                    stot = p3.tile([128, c.VC], f32, tag="stot")
                    nc.vector.tensor_tensor(out=stot[:], in0=s01[:], in1=s23[:],
                                            op=AOP.add)
                    hb2 = p3.tile([128, 128], b16, tag="hb2")
                    nc.sync.dma_start(out=hb2[:rows, :],
                                      in_=h_slice[t * 128:t * 128 + rows, 0:128])
                    dmx = p3.tile([128, 4], f32, tag="dmx")
                    nc.vector.tensor_scalar_max(dmx[:], stot[:, 128:132], 1e-30)
                    rec = p3.tile([128, 4], f32, tag="rec")
                    nc.vector.reciprocal(out=rec[:], in_=dmx[:])
                    tmp = p3.tile([128, 128], f32, tag="tmp")
                    nm = stot[:, 0:128].rearrange("p (f h) -> p f h", h=4)
                    rc = rec[:].unsqueeze(1).broadcast_to([128, 32, 4])
                    tw = tmp[:].rearrange("p (h f) -> p f h", h=4)
                    nc.vector.tensor_tensor(out=tw, in0=nm, in1=rc, op=AOP.mult)
                    ot = p3.tile([128, 128], f32, tag="ot")
                    hperm = hb2[:].rearrange("p (f h) -> p h f", h=4)
                    o2 = ot[:].rearrange("p (h f) -> p h f", h=4)
                    t2 = tmp[:].rearrange("p (h f) -> p h f", h=4)
                    nc.vector.tensor_tensor(out=o2, in0=t2, in1=hperm, op=AOP.add)
                    nc.sync.dma_start(out=out_ext[t * 128:t * 128 + rows, :],
                                      in_=ot[:rows, :])

    lower_extended_insts(nc)
    return nc


def wrap16(arr, n):
    w = np.zeros((128, n // 16), arr.dtype)
    w[:16, :] = arr.reshape(n // 16, 16).T
    w[16:, :] = np.tile(w[:16, :], (7, 1))
    return w


def host_prep(cfg, node_features, edge_index, W, b, attn):
    c = cfg
    X = np.asarray(node_features, np.float32)
    W = np.asarray(W, np.float32)
    bb = np.asarray(b, np.float32)
    attn = np.asarray(attn, np.float32)
    tgt = np.asarray(edge_index[0], np.int64)
    src = np.asarray(edge_index[1], np.int64)

    H, FH = c.H, c.FH
    a_i, a_j = attn[0, :, :FH], attn[0, :, FH:]
    Wp = W.reshape(c.FIN, H, FH).transpose(0, 2, 1).reshape(c.FIN, H * FH)
    bp = bb.reshape(H, FH).T.reshape(-1)
    WA_i = np.einsum('ihf,hf->ih', W.reshape(c.FIN, H, FH), a_i)
    WA_j = np.einsum('ihf,hf->ih', W.reshape(c.FIN, H, FH), a_j)
    bA_i = np.einsum('hf,hf->h', bb.reshape(H, FH), a_i)
    bA_j = np.einsum('hf,hf->h', bb.reshape(H, FH), a_j)
    waug = np.concatenate([Wp, WA_i, WA_j], axis=1).astype(np.float32)
    baug = np.concatenate([bp, bA_i, bA_j])[None, :].astype(np.float32)

    core_of = tgt // c.NPC
    NW = (c.NPC + 127) // 128  # fixed 128-node windows
    ecount = np.zeros((c.ncores, c.NPASS, NW), np.int64)
    edges = {}
    for cc in range(c.ncores):
        m = core_of == cc
        tl_g = (tgt[m] - cc * c.NPC).astype(np.int64)
        sr = src[m]
        o = np.argsort(tl_g, kind='stable')
        tl_g, sr = tl_g[o], sr[o]
        for p in range(c.NPASS):
            pm = (sr // c.SCHUNK) == p
            tlp, srp = tl_g[pm], sr[pm] - p * c.SCHUNK
            edges[(cc, p)] = (tlp, srp)
            ecount[cc, p] = np.bincount(tlp // 128, minlength=NW)
    # per-window tile budget shared across cores (max), then pack into batches
    T = np.maximum(ecount, 0).max(axis=0)          # [NPASS, NW] edges max
    T = (T + 127) // 128                            # tiles per window
    batches = []   # list of (pass, [(w, ntiles), ...])
    for p in range(c.NPASS):
        cur, cnt = [], 0
        for w in range(NW):
            tw = int(T[p, w])
            if tw == 0:
                continue
            if cnt + tw > c.K:
                batches.append((p, cur)); cur, cnt = [], 0
            cur.append((w, tw)); cnt += tw
        if cur:
            batches.append((p, cur))
    NB = len(batches)

    plan = {"NB": NB, "pass_of": [p for p, _ in batches], "nidx": [], "chunks": []}
    for p, wins in batches:
        ntile = sum(tw for _, tw in wins)
        plan["nidx"].append(min((ntile * 128 + 2047) // 2048 * 2048, c.EPB))
        chl = []
        for w, tw in wins:
            base = w * 128
            wrows = min(128, c.NPC - base)
            chl.append((base, wrows, tw))
        plan["chunks"].append(chl)

    in_maps = []
    for cc in range(c.ncores):
        g_i = np.zeros((NB, c.EPB), np.int16)
        t_l = np.full((NB, c.EPB), 999.0, np.float32)
        for bi, (p, wins) in enumerate(batches):
            tlp, srp = edges[(cc, p)]
            cur = 0
            for w, tw in wins:
                i0 = int(np.searchsorted(tlp, w * 128, 'left'))
                i1 = int(np.searchsorted(tlp, w * 128 + 128, 'left'))
                ln = i1 - i0
                g_i[bi, cur:cur + ln] = srp[i0:i1].astype(np.int16)
                t_l[bi, cur:cur + ln] = (tlp[i0:i1] - w * 128).astype(np.float32)
                cur += tw * 128
        g_w = np.stack([wrap16(g_i[bn], c.EPB) for bn in range(NB)])
        tl_w = t_l.reshape(NB, c.K, 128).transpose(0, 2, 1).astype(bf16)
        tlr_w = t_l.reshape(NB, 1, c.EPB).astype(bf16)
        xtv = np.zeros((c.FIN, c.NT1 * 128), np.float32)
        xtv[:, :c.NPC] = X[cc * c.NPC:(cc + 1) * c.NPC].T
        in_maps.append({
            "xt": xtv, "waug": waug, "baug": baug,
            "g_idx": g_w, "tl": tl_w, "tlr": tlr_w,
        })
    return in_maps, plan, edges


def reference_np(cfg, node_features, edge_index, W, b, attn):
    c = cfg
    X = np.asarray(node_features, np.float64)
    W_ = np.asarray(W, np.float64)
    b_ = np.asarray(b, np.float64)
    a = np.asarray(attn, np.float64)
    h = (X @ W_ + b_).reshape(c.N, c.H, c.FH)
    tgt, src = np.asarray(edge_index[0]), np.asarray(edge_index[1])
    a_i, a_j = a[0, :, :c.FH], a[0, :, c.FH:]
    s_i = np.einsum('nhf,hf->nh', h, a_i)
    s_j = np.einsum('nhf,hf->nh', h, a_j)
    e = s_i[tgt] + s_j[src]
    e = np.where(e > 0, e, c.NEG * e)
    m = np.full((c.N, c.H), -np.inf)
    np.maximum.at(m, tgt, e)
    ex = np.exp(e - m[tgt])
    den = np.zeros((c.N, c.H))
    np.add.at(den, tgt, ex)
    alpha = ex / den[tgt]
    msg = alpha[:, :, None] * h[src]
    out = np.zeros((c.N, c.H, c.FH))
    np.add.at(out, tgt, msg)
    out = out + h
    return out.reshape(c.N, c.H * c.FH).astype(np.float32)


# ---------------- public entry point ----------------
import os as _os

LAST_EXEC_NS = None


def kernel(**inputs):
    global LAST_EXEC_NS
    from concourse.bass_utils import run_bass_kernel_spmd

    cfg = Cfg()
    node_features = inputs["node_features"]
    edge_index = inputs["edge_index"]
    W, b, attn = inputs["W"], inputs["b"], inputs["attn"]

    in_maps, plan, _ = host_prep(cfg, node_features, edge_index, W, b, attn)
    nc = build_nc(cfg, plan)
    res = run_bass_kernel_spmd(
        nc, in_maps, list(range(cfg.ncores)),
        trace=bool(_os.environ.get("GAT_TRACE")),
    )
    LAST_EXEC_NS = res.exec_time_ns
    out = np.concatenate(
        [np.asarray(res.results[i]["out"]) for i in range(cfg.ncores)], axis=0)
    return np.ascontiguousarray(out, dtype=np.float32)
